# revision 41
# baseline (speedup 1.0000x reference)
"""GQA dense-transformer block (RMSNorm + QKV + RoPE + causal GQA attention
+ o_proj + residual) on 8 trn2 NeuronCores.

Sharding: 2 (batch) x 4 (head-group tensor parallel). Core c = 4*b + g handles
batch b, q-heads 8g..8g+7, kv-heads 2g..2g+1. Each core produces a partial
o_proj output (+ the RMS-normed residual, supplied pre-multiplied by rms_w on
g==0 cores and as zeros elsewhere); the host sums the 4 partials per batch.

Kernel design (measured ~370us/core vs the 1.49ms fp32 ancestor):
- All matmuls in bf16 (PE: 1 cycle/row vs fp32's 4); PSUM accumulates fp32.
- Feature-major attention: projections produce Q^T/K^T ([head_dim, tokens]) so
  scores come out transposed ([k, q]) and softmax needs no transpose. exp has
  no max-subtraction (scores are O(5)); the denominator comes from a
  ones-column appended to V (PV matmul M=65); the division is deferred one
  window and applied as a one-hot selector matmul broadcast + in-place
  multiply on the raw numerators parked in AT.
- Per-token rsqrt(mean x^2) via a ones-column matmul over ACT-squared x^T
  chunks; rsqrt/reciprocals run on ACT as exp(-a*ln(x)) (single table set, and
  the serial 1-lane DVE reciprocal stays off every critical path).
- Causal structure exploited twice: fully-masked key blocks are skipped, and
  diagonal blocks are trimmed to the live column range [dd, 512) in the score
  matmul, exp, mask multiply, and PV matmul.
- Engine overlap: attention for windows 0-1 is emitted inside phase P (its exp
  hides under projection matmuls); o_proj token-tiles are interleaved into the
  exp-bound tail of windows 2-3 as soon as their window's AT is normalized.
- PSUM: 8 banks exactly, per phase. A matmul with start=True zeroes the WHOLE
  2KB bank, so when several column-slices of one bank accumulate independently
  (the V subtiles), only the very first matmul may carry start=True.
"""

import math
import numpy as np

# model dims (hardcoded per contract)
B, S, D = 2, 2048, 2048
HQ, HKV, HD = 32, 8, 64
NC = 8
NG = 4            # head groups
QH = 8            # q heads per core
KH = 2            # kv heads per core
CQ = QH * HD      # 512 q cols per core
W512 = S // 512   # 4 token windows
NT = S // 128     # 16 token tiles
NDC = D // 128    # 16 contraction chunks
PERM = [0, 4, 1, 5, 2, 6, 3, 7]  # local head order: ptile p = (h=p | h=p+4)

_cache = {}
_patched = [False]


def _legalize_bir_bytes(bir):
    """Walrus in this container accepts at most ONE embedded sem-wait per TPB
    instruction ("Too many sync wait commands"). Tile emits several when an
    instruction depends on multiple DMA queues. Split the extras into
    standalone EventSemaphore (pure-wait) instructions on the same engine
    immediately before the instruction — identical blocking semantics."""
    import json
    d = json.loads(bir if isinstance(bir, str) else bir.decode())
    n_split = 0
    stack = [d]
    while stack:
        o = stack.pop()
        if isinstance(o, dict):
            insts = o.get("instructions")
            if isinstance(insts, list) and insts and isinstance(insts[0], dict) \
               and "opcode" in insts[0]:
                new = []
                for inst in insts:
                    si = inst.get("sync_info") or {}
                    ws = si.get("on_wait") or []
                    if len(ws) > 1 and isinstance(inst.get("opcode"), str) \
                       and inst.get("opcode") not in (
                            "EventSemaphore", "UnconditionalBranch",
                            "Call", "ISA"):
                        for k, w in enumerate(ws[:-1]):
                            n_split += 1
                            new.append({
                                "debug": inst.get("debug", 0),
                                "engine": inst["engine"],
                                "ins": [], "outs": [],
                                "name": f"lw{n_split}_{inst['name']}",
                                "opcode": "EventSemaphore",
                                "sync_info": {"on_update": [], "on_wait": [w]},
                            })
                        si["on_wait"] = [ws[-1]]
                    new.append(inst)
                o["instructions"] = new
            else:
                stack.extend(o.values())
        elif isinstance(o, list):
            stack.extend(o)
    return json.dumps(d).encode()


def _install_patch():
    if _patched[0]:
        return
    from concourse import bass_utils as bu
    from concourse import bass2jax as b2j
    orig = bu.compile_bir_kernel

    def patched(bir, *a, **k):
        return orig(_legalize_bir_bytes(bir), *a, **k)

    bu.compile_bir_kernel = patched
    b2j.compile_bir_kernel = patched
    _patched[0] = True


def _build(causal: bool):
    import concourse.bass as bass
    import concourse.mybir as mybir
    from concourse.tile import TileContext

    fp32 = mybir.dt.float32
    bf16 = mybir.dt.bfloat16
    AF = mybir.ActivationFunctionType

    nc = bass.Bass("TRN2")
    # host-prepped inputs (see _host_prep for layouts)
    xt = nc.dram_tensor("xt", [128, W512, NDC, 512], bf16, kind="ExternalInput")
    xr = nc.dram_tensor("xr", [S, D], fp32, kind="ExternalInput")
    wq = nc.dram_tensor("wq", [128, NDC, CQ], bf16, kind="ExternalInput")
    wk = nc.dram_tensor("wk", [128, NDC, KH * HD], bf16, kind="ExternalInput")
    wv = nc.dram_tensor("wv", [128, NDC, KH * HD], bf16, kind="ExternalInput")
    wo = nc.dram_tensor("wo", [128, 4, D], bf16, kind="ExternalInput")
    cosT_d = nc.dram_tensor("cosT", [128, S], fp32, kind="ExternalInput")
    sinT_d = nc.dram_tensor("sinT", [128, S], fp32, kind="ExternalInput")
    maskb_d = nc.dram_tensor("maskb", [128, 2, 896], bf16, kind="ExternalInput")
    sel8_d = nc.dram_tensor("sel8", [97, 2, 128], bf16, kind="ExternalInput")
    out = nc.dram_tensor("out", [S, D], fp32, kind="ExternalOutput")

    with TileContext(nc) as tc:
        with (
            tc.tile_pool(name="res", bufs=1) as res,
            tc.tile_pool(name="dram", bufs=1, space="DRAM") as dpool,
        ):
            # resident tiles
            QT = [res.tile([128, S], bf16, tag=f"qt{p}", name=f"qt{p}") for p in range(4)]
            KT = res.tile([128, S], bf16, tag="kt", name="kt")
            AT = [res.tile([128, S], bf16, tag=f"at{p}", name=f"at{p}") for p in range(4)]
            v_all = res.tile([128, NT * 130], bf16, tag="vall", name="vall")
            cosT = res.tile([128, S], fp32, tag="cosT")
            sinT = res.tile([128, S], fp32, tag="sinT")
            maskb = res.tile([128, 2, 896], bf16, tag="maskb", name="maskb")
            wq_sb = res.tile([128, NDC, CQ], bf16, tag="wqsb", name="wq_sb")
            wk_sb = res.tile([128, NDC, KH * HD], bf16, tag="wksb", name="wk_sb")
            wv_sb = res.tile([128, NDC, KH * HD], bf16, tag="wvsb", name="wv_sb")
            wo_sb = res.tile([128, 4, D], bf16, tag="wosb", name="wo_sb")
            s_all = res.tile([128, NT], fp32, tag="sall", name="s_all")
            ones_col = res.tile([128, 1], bf16, tag="onesc", name="ones_col")
            ones1f = res.tile([1, 128], fp32, tag="ones1f", name="ones1f")
            sel8 = res.tile([97, 2, 128], bf16, tag="sel8", name="sel8")
            dens = [res.tile([97, 512], fp32, tag=f"den{i}", name=f"den{i}")
                    for i in range(4)]
            invs = [res.tile([97, 512], bf16, tag=f"inv{i}", name=f"inv{i}")
                    for i in range(4)]
            epst = res.tile([1, 1], fp32, tag="epst", name="epst")
            s_dram = dpool.tile([S, 1], fp32, tag="sdram", name="s_dram")

            nc.vector.memset(ones_col[:, :], 1.0)
            nc.vector.memset(ones1f[:, :], 1.0)
            # sel8[:, p, :] is the one-hot selector that broadcasts inv row
            # 2p+h to output partitions 64h..64h+63 in one matmul
            nc.vector.memset(epst[:, :], float(np.finfo(np.float32).eps))
            warmt = res.tile([1, 1], fp32, tag="warmt", name="warmt")
            nc.scalar.activation(out=warmt[:, :], in_=epst[:, :], func=AF.Exp)
            for i in range(4):
                nc.vector.memset(dens[i][:, :], 1.0)
            for tt in range(NT):
                nc.vector.memset(v_all[:, 130 * tt + 64 : 130 * tt + 65], 1.0)
                nc.vector.memset(v_all[:, 130 * tt + 129 : 130 * tt + 130], 1.0)
            nc.sync.dma_start(out=wq_sb[:, :, :], in_=wq[:, :, :])
            nc.sync.dma_start(out=wk_sb[:, :, :], in_=wk[:, :, :])
            nc.sync.dma_start(out=wv_sb[:, :, :], in_=wv[:, :, :])
            nc.sync.dma_start(out=cosT[:, :], in_=cosT_d[:, :])
            nc.sync.dma_start(out=sinT[:, :], in_=sinT_d[:, :])
            nc.sync.dma_start(out=sel8[:, :, :], in_=sel8_d[:, :, :])
            nc.sync.dma_start(out=maskb[:, :, :], in_=maskb_d[:, :, :])
            nc.sync.dma_start(out=wo_sb[:, :, :], in_=wo[:, :, :])

            # ---- attention-window emitter (pools passed per call-site) ----
            def emit_A_p(w, p, psc, ppv, aex):
                kt_max = 4 * (w + 1) if causal else NT
                wsl = slice(512 * w, 512 * (w + 1))
                # AT holds raw PV numerators; denominators collect at
                # 32-aligned partitions (DVE write rule) so one reciprocal
                # covers 4, then a one-hot selector matmul broadcasts both
                # heads' inv rows and AT is normalized in place (deferred).
                if True:
                    pvs = [ppv.tile([65, 512], fp32, tag="pv", name="pv")
                           for _ in range(2)]
                    for kt in range(kt_max):
                        dd = 128 * kt - 512 * w
                        # diagonal blocks: columns q < dd are fully masked and
                        # never read downstream — trim scores/exp/mask/PV to
                        # the live range [dd, 512)
                        t0 = dd if (causal and 0 < dd <= 384) else 0
                        sc = psc.tile([128, 2, 512], fp32, tag="sc", name="sc")
                        for h in range(2):
                            nc.tensor.matmul(
                                sc[:, h, t0:],
                                KT[64 * h : 64 * (h + 1), kt * 128 : (kt + 1) * 128],
                                QT[p][64 * h : 64 * (h + 1),
                                      512 * w + t0 : 512 * (w + 1)],
                                start=True, stop=True)
                        ex = aex.tile([128, 2, 512], bf16, tag="ex", name="ex")
                        nc.scalar.activation(out=ex[:, :, t0:], in_=sc[:, :, t0:],
                                             func=AF.Exp)
                        if causal and 0 <= dd <= 384:
                            nc.vector.tensor_mul(ex[:, :, t0:], ex[:, :, t0:],
                                                 maskb[:, :, 384 : 896 - dd])
                        for h in range(2):
                            nc.tensor.matmul(
                                pvs[h][:, t0:],
                                v_all[:, 130 * kt + 65 * h : 130 * kt + 65 * (h + 1)],
                                ex[:, h, t0:],
                                start=(kt == 0), stop=(kt == kt_max - 1),
                                skip_group_check=True)
                    for h in range(2):
                        nc.vector.tensor_scalar_mul(
                            AT[p][64 * h : 64 * (h + 1), wsl],
                            pvs[h][0:64, :], 1.0)
                        r = 64 * (p % 2) + 32 * h
                        nc.vector.tensor_scalar_mul(
                            dens[2 * (w % 2) + p // 2][r : r + 1, :],
                            pvs[h][64:65, :], 1.0)
            def emit_recips(w, asq, recip_on_act=True):
                for half in range(2):
                    if recip_on_act:
                        lnd = asq.tile([97, 512], fp32, tag="lnd", name="lnd")
                        nc.scalar.activation(out=lnd[:, :],
                                             in_=dens[2 * (w % 2) + half][:, :],
                                             func=AF.Ln)
                        nc.scalar.activation(out=invs[2 * (w % 2) + half][:, :],
                                             in_=lnd[:, :], func=AF.Exp, scale=-1.0)
                    else:
                        with nc.allow_low_precision("softmax denom bcast bf16"):
                            nc.vector.reciprocal(
                                out=invs[2 * (w % 2) + half][:, :],
                                in_=dens[2 * (w % 2) + half][:, :])

            def emit_A(w, psc, ppv, aex, asq, recip_on_act=True):
                for p in range(4):
                    emit_A_p(w, p, psc, ppv, aex)
                emit_recips(w, asq, recip_on_act)

            def apply_div(w, ppv):
                wsl_ = slice(512 * w, 512 * (w + 1))
                for p in range(4):
                    bcp = ppv.tile([128, 512], fp32, tag="pv", name="bcp")
                    nc.tensor.matmul(bcp[:, :], sel8[:, p % 2, :],
                                     invs[2 * (w % 2) + p // 2][:, :],
                                     start=True, stop=True)
                    nc.vector.tensor_mul(AT[p][:, wsl_], AT[p][:, wsl_],
                                         bcp[:, :])

            # ---- phase P (+ windows 0-1 of attention hidden under it) ----
            with (
                tc.tile_pool(name="ps_acc", bufs=2, space="PSUM") as pacc,
                tc.tile_pool(name="ps_st", bufs=1, space="PSUM") as pst,
                tc.tile_pool(name="ps_v", bufs=1, space="PSUM") as pvv,
                tc.tile_pool(name="ps_sm", bufs=1, space="PSUM") as psc_m,
                tc.tile_pool(name="ps_pm", bufs=2, space="PSUM") as ppv_m,
                tc.tile_pool(name="xw_p", bufs=3) as xwp,
                tc.tile_pool(name="sq_p", bufs=3) as sqp,
                tc.tile_pool(name="sw_p", bufs=2) as swp,
                tc.tile_pool(name="cf_p", bufs=2) as cfp,
                tc.tile_pool(name="rt_p", bufs=2) as rtp,
                tc.tile_pool(name="aex_m", bufs=4) as aex_m,
            ):
                def emit_P(w):
                    wsl = slice(512 * w, 512 * (w + 1))
                    xw = xwp.tile([128, NDC, 512], bf16, tag="xw", name="xw")
                    # spread the window load across all three DMA-capable
                    # queues so the first chunks land ~3x sooner
                    for dcg, eng in enumerate((nc.gpsimd, nc.sync, nc.scalar,
                                               nc.gpsimd)):
                        eng.dma_start(
                            out=xw[:, 4 * dcg : 4 * (dcg + 1), :],
                            in_=xt[:, w, 4 * dcg : 4 * (dcg + 1), :])

                    # per-token 1/sqrt(mean(x^2)+eps) via ones-column matmul
                    # (stats/psb get their own PSUM tag so the serial s chain
                    # never stalls the Q/K accumulation ring)
                    ps_st = pst.tile([1, 512], fp32, tag="st", name="ps_st")
                    for dcg in range(8):
                        sq = sqp.tile([128, 2, 512], bf16, tag="sq", name="sq")
                        nc.scalar.activation(out=sq[:, :, :],
                                             in_=xw[:, 2 * dcg : 2 * (dcg + 1), :],
                                             func=AF.Square)
                        for j in range(2):
                            dc = 2 * dcg + j
                            nc.tensor.matmul(ps_st[:, :], ones_col[:, :], sq[:, j, :],
                                             start=(dc == 0), stop=(dc == NDC - 1))
                    lnm = swp.tile([1, 512], fp32, tag="sqs", name="lnm")
                    nc.scalar.activation(out=lnm[:, :], in_=ps_st[:, :], func=AF.Ln,
                                         bias=epst[:, 0:1], scale=1.0 / D)
                    s_w = swp.tile([1, 512], fp32, tag="sw", name="s_w")
                    nc.scalar.activation(out=s_w[:, :], in_=lnm[:, :], func=AF.Exp,
                                         scale=-0.5)
                    # partition-layout copy of s for V scaling + residual
                    nc.gpsimd.dma_start(
                        out=s_dram[wsl, :].rearrange("s one -> one s"),
                        in_=s_w[0:1, :])
                    nc.gpsimd.dma_start(
                        out=s_all[:, 4 * w : 4 * w + 4],
                        in_=s_dram[wsl, :].rearrange("(t p) one -> p (t one)", p=128))
                    # broadcast s to 128 partitions; fold into rope tables
                    psb = pst.tile([128, 512], fp32, tag="st", name="psb")
                    nc.tensor.matmul(psb[:, :], ones1f[0:1, :], s_w[0:1, :],
                                     start=True, stop=True)
                    cosF = cfp.tile([128, 512], bf16, tag="cosF", name="cosF")
                    sinF = cfp.tile([128, 512], bf16, tag="sinF", name="sinF")
                    nc.vector.tensor_mul(cosF[:, :], cosT[:, wsl], psb[:, :])
                    nc.vector.tensor_mul(sinF[:, :], sinT[:, wsl], psb[:, :])

                    # Q (4 ptiles) + K projections, feature-major, with RoPE
                    for ct in range(5):
                        ps = pacc.tile([128, 512], fp32, tag="acc", name="ps_qk")
                        for dc in range(NDC):
                            lhs = (wq_sb[:, dc, 128 * ct : 128 * (ct + 1)] if ct < 4
                                   else wk_sb[:, dc, :])
                            nc.tensor.matmul(ps[:, :], lhs, xw[:, dc, :],
                                             start=(dc == 0), stop=(dc == NDC - 1))
                        dst = KT if ct == 4 else QT[ct]
                        tmp = rtp.tile([128, 512], bf16, tag="rt", name="rt")
                        for a, bidx in ((0, 1), (1, 0), (2, 3), (3, 2)):
                            nc.vector.tensor_mul(tmp[32 * a : 32 * (a + 1), :],
                                                 ps[32 * bidx : 32 * (bidx + 1), :],
                                                 sinF[32 * a : 32 * (a + 1), :])
                        nc.vector.tensor_mul(dst[:, wsl], ps[:, :], cosF[:, :])
                        nc.vector.tensor_add(dst[:, wsl], dst[:, wsl], tmp[:, :])

                    # V projection, token-major. One PSUM bank accumulates all 4
                    # token-subtiles: only the first matmul may set start=True
                    # (start zeroes the whole bank).
                    vs = pvv.tile([128, 512], fp32, tag="psv", name="psv")
                    for vt in range(4):
                        for dc in range(NDC):
                            nc.tensor.matmul(
                                vs[:, 128 * vt : 128 * (vt + 1)],
                                xw[:, dc, 128 * vt : 128 * (vt + 1)],
                                wv_sb[:, dc, :],
                                start=(vt == 0 and dc == 0),
                                stop=(dc == NDC - 1),
                                skip_group_check=True)
                        tt = 4 * w + vt
                        for h in range(KH):
                            nc.vector.tensor_scalar_mul(
                                v_all[:, 130 * tt + 65 * h : 130 * tt + 65 * h + 64],
                                vs[:, 128 * vt + 64 * h : 128 * vt + 64 * (h + 1)],
                                s_all[:, tt : tt + 1])

                emit_P(0)
                emit_P(1)
                emit_A(0, psc_m, ppv_m, aex_m, swp)
                emit_P(2)
                emit_A(1, psc_m, ppv_m, aex_m, swp)
                emit_P(3)

            # ---- phase A: remaining windows with full pools ----
            with (
                tc.tile_pool(name="ps_s", bufs=3, space="PSUM") as psc,
                tc.tile_pool(name="ps_pv", bufs=2, space="PSUM") as ppv,
                tc.tile_pool(name="aex", bufs=6) as aex,
                tc.tile_pool(name="asq", bufs=2) as asq,
            ):
                apply_div(0, ppv)
                emit_A(2, psc, ppv, aex, asq)
                apply_div(1, ppv)
                emit_A(3, psc, ppv, aex, asq, recip_on_act=False)
                apply_div(2, ppv)
                apply_div(3, ppv)

            # ---- phase O: o_proj + scaled residual ----
            with (
                tc.tile_pool(name="ps_o", bufs=4, space="PSUM") as po,
                tc.tile_pool(name="oxp", bufs=6) as oxp,
                tc.tile_pool(name="oep", bufs=4) as oep,
            ):
                for dw in range(4):
                    dsl = slice(512 * dw, 512 * (dw + 1))
                    for tt in range(NT):
                        pso = po.tile([128, 512], fp32, tag="pso", name="pso")
                        for c in range(4):
                            nc.tensor.matmul(pso[:, :],
                                             AT[c][:, tt * 128 : (tt + 1) * 128],
                                             wo_sb[:, c, dsl],
                                             start=(c == 0), stop=(c == 3))
                        x_s = oxp.tile([128, 512], fp32, tag="xs2", name="xs2")
                        nc.gpsimd.dma_start(out=x_s[:, :],
                                            in_=xr[tt * 128 : (tt + 1) * 128, dsl])
                        xn = oep.tile([128, 512], fp32, tag="xn", name="xn")
                        nc.scalar.activation(out=xn[:, :], in_=x_s[:, :], func=AF.Copy,
                                             scale=s_all[:, tt : tt + 1])
                        ob = oep.tile([128, 512], fp32, tag="ob", name="ob")
                        nc.vector.tensor_add(ob[:, :], xn[:, :], pso[:, :])
                        nc.sync.dma_start(out=out[tt * 128 : (tt + 1) * 128, dsl],
                                          in_=ob[:, :])
    return nc


def _host_prep(x, rms_w, Wq, Wk, Wv, Wo):
    import ml_dtypes
    f32 = np.float32
    bf = ml_dtypes.bfloat16
    x = np.asarray(x, f32)
    rms_w = np.asarray(rms_w, f32)
    wq_full = (np.asarray(Wq, f32) * rms_w[:, None] / math.sqrt(HD)).astype(f32)
    wk_full = (np.asarray(Wk, f32) * rms_w[:, None]).astype(f32)
    wv_full = (np.asarray(Wv, f32) * rms_w[:, None]).astype(f32)
    Wo = np.asarray(Wo, f32)

    inv_f = (1.0 / (10000.0 ** (np.arange(0, HD, 2, dtype=f32) / HD))).astype(f32)
    freqs = np.arange(S, dtype=f32)[:, None] * inv_f[None, :]   # [S, 32]
    cos = np.cos(freqs).astype(f32).T                           # [32, S]
    sin = np.sin(freqs).astype(f32).T
    cosT = np.tile(np.concatenate([cos, cos], 0), (2, 1))       # [128, S]
    sinT = np.tile(np.concatenate([-sin, sin], 0), (2, 1))

    kk = np.arange(128)[:, None]
    jj = np.arange(896)[None, :]
    maskb1 = (jj >= kk + 384).astype(bf)
    maskb = np.ascontiguousarray(np.stack([maskb1, maskb1], axis=1))  # [128,2,896]
    sel8 = np.zeros((97, 2, 128), bf)
    for j in range(2):
        for h in range(2):
            sel8[64 * j + 32 * h, j, 64 * h : 64 * (h + 1)] = 1

    def chunked(wfull):
        # [D, C] -> [128, NDC, C]: row 128*dc+p goes to [p, dc, :]
        C = wfull.shape[1]
        return np.ascontiguousarray(
            wfull.reshape(NDC, 128, C).transpose(1, 0, 2)).astype(bf)

    zeros_xr = np.zeros((S, D), f32)
    per_core = []
    for c in range(NC):
        b, g = c // 4, c % 4
        heads = [8 * g + h for h in PERM]
        wq_g = np.concatenate([wq_full[:, 64 * h : 64 * (h + 1)] for h in heads], axis=1)
        wo_g = np.concatenate([Wo[64 * h : 64 * (h + 1), :] for h in heads], axis=0)
        wk_g = wk_full[:, 128 * g : 128 * (g + 1)]
        wv_g = wv_full[:, 128 * g : 128 * (g + 1)]
        xT = x[b].T.astype(bf)                                   # [D, S]
        # [128, W512, NDC, 512]: xt[p, w, dc, s] = xT[128*dc+p, 512*w+s]
        xt_r = np.ascontiguousarray(
            xT.reshape(NDC, 128, W512, 512).transpose(1, 2, 0, 3))
        wo_r = np.ascontiguousarray(
            wo_g.reshape(4, 128, D).transpose(1, 0, 2)).astype(bf)  # [128, 4, D]
        xr_g = np.ascontiguousarray(x[b] * rms_w[None, :]) if g == 0 else zeros_xr
        per_core.append({
            "xt": xt_r, "xr": xr_g,
            "wq": chunked(wq_g), "wk": chunked(wk_g), "wv": chunked(wv_g),
            "wo": wo_r,
            "cosT": np.ascontiguousarray(cosT), "sinT": np.ascontiguousarray(sinT),
            "maskb": maskb, "sel8": sel8,
        })
    return per_core


def kernel(x, rms_w, Wq, Wk, Wv, Wo, apply_causal_mask, _trace=False):
    from concourse import bass_utils
    _install_patch()
    causal = bool(int(np.asarray(apply_causal_mask)))
    if causal not in _cache:
        _cache[causal] = _build(causal)
    nc = _cache[causal]
    in_maps = _host_prep(x, rms_w, Wq, Wk, Wv, Wo)
    r = bass_utils.run_bass_kernel_spmd(nc, in_maps, core_ids=list(range(NC)),
                                        trace=_trace)
    outs = [r.results[c]["out"] for c in range(NC)]
    full = np.stack([outs[4 * b] + outs[4 * b + 1] + outs[4 * b + 2] + outs[4 * b + 3]
                     for b in range(B)]).astype(np.float32)
    if _trace:
        kernel.last_exec_time_ns = r.exec_time_ns
    return full
